# revision 1
# baseline (speedup 1.0000x reference)
"""Trainium2 Bass kernel for the GCN message-passing block (nn_Model_16217796510271).

Contract: kernel(**inputs) takes the FULL fp32 inputs (x: [64,243,17,256] plus
weights) and returns the FULL fp32 output [64,243,17,256]. Internally the batch
axis is sharded 8 ways across NeuronCores; BatchNorm statistics are combined
with an on-device AllReduce.

Per-core layout: channels-on-partitions ("transposed") — xt[c, j, bt] with
C=256 split into two 128-partition chunks. V/U/att1 matmuls contract over C on
the PE (bf16 inputs, fp32 PSUM). The 17x17 normalized-adjacency mix is a small
set of fused scalar*tensor+tensor AXPY ops on the Vector engine over per-joint
column blocks. BatchNorm is two-pass: pass 1 accumulates per-joint sum/sumsq
(fused accum_out reductions), a [1,34] AllReduce combines cores, pass 2
recomputes y and applies BN + residual ReLU + the joint attention gate.
"""

import sys

for _p in ("/opt/trn_rl_repo",):
    if _p not in sys.path:
        sys.path.insert(0, _p)

import ml_dtypes
import numpy as np

import concourse.bacc as bacc
import concourse.bass as bass
import concourse.tile as tile
from concourse import bass_isa, mybir
from concourse.bass_utils import run_bass_kernel_spmd

# ---------------------------------------------------------------- problem constants
CONNECTIONS = {
    10: [9], 9: [8, 10], 8: [7, 9], 14: [15, 8], 15: [16, 14], 11: [12, 8],
    12: [13, 11], 7: [0, 8], 0: [1, 7], 1: [2, 0], 2: [3, 1], 4: [5, 0],
    5: [6, 4], 16: [15], 13: [12], 3: [2], 6: [5],
}
J = 17
C = 256
H = 64          # attention hidden
B = 64
T = 243
EPS = 1e-5

NCORES = 8
BPC = B // NCORES            # batches per core
NBT = BPC * T                # 1944 (b,t) columns per core
W = 243                      # window width in (b,t) columns
NW = NBT // W                # 8 windows
NGLOB = B * T * C            # BN normalization count per joint

F32 = mybir.dt.float32
BF16 = mybir.dt.bfloat16


def _norm_adj() -> np.ndarray:
    adj = np.zeros((J, J), dtype=np.float32)
    for i, ks in CONNECTIONS.items():
        for k in ks:
            adj[i, k] = 1.0
    dinv = adj.sum(-1) ** -0.5
    return (dinv[:, None] * adj * dinv[None, :]).astype(np.float32)


_ADJ = _norm_adj()


# ---------------------------------------------------------------- device program
def _build_program() -> bass.Bass:
    nc = bacc.Bacc(
        "TRN2",
        target_bir_lowering=False,
        debug=False,
        num_devices=NCORES,
    )

    # I/O (per core)
    xt = nc.dram_tensor("xt", [NW, 128, 2, J, W], F32, kind="ExternalInput").ap()
    wv = nc.dram_tensor("wv", [2, 2, 128, 128], BF16, kind="ExternalInput").ap()
    wu = nc.dram_tensor("wu", [2, 2, 128, 128], BF16, kind="ExternalInput").ap()
    wa1 = nc.dram_tensor("wa1", [2, 128, H], BF16, kind="ExternalInput").ap()
    wa2 = nc.dram_tensor("wa2", [H, 1], BF16, kind="ExternalInput").ap()
    bias2 = nc.dram_tensor("bias2", [C, J], F32, kind="ExternalInput").ap()
    bnw = nc.dram_tensor("bnw", [1, J], F32, kind="ExternalInput").ap()
    bnb = nc.dram_tensor("bnb", [1, J], F32, kind="ExternalInput").ap()
    ab1 = nc.dram_tensor("ab1", [H, 1], F32, kind="ExternalInput").ap()
    ab2 = nc.dram_tensor("ab2", [1, 1], F32, kind="ExternalInput").ap()
    out_t = nc.dram_tensor("out_t", [NW, 128, 2, J, W], F32, kind="ExternalOutput").ap()

    bias2v = bias2.rearrange("(q p) j -> p q j", q=2)      # [128, 2, J]

    with tile.TileContext(nc) as tc:
        with (
            tc.tile_pool(name="consts", bufs=1) as consts,
            tc.tile_pool(name="xbfp", bufs=2) as xbfp,
            tc.tile_pool(name="xfp", bufs=2) as xfp,
            tc.tile_pool(name="vxp", bufs=4, space="PSUM") as vxp,
            tc.tile_pool(name="uxp", bufs=2, space="PSUM") as uxp,
            tc.tile_pool(name="vxs", bufs=2 * J + 2) as vxs,
            tc.tile_pool(name="hbfp", bufs=4) as hbfp,
            tc.tile_pool(name="ytmp", bufs=8) as ytmp,
            tc.tile_pool(name="ybp", bufs=4) as ybp,
            tc.tile_pool(name="ysqp", bufs=3) as ysqp,
            tc.tile_pool(name="osbp", bufs=6) as osbp,
            tc.tile_pool(name="obfp", bufs=4) as obfp,
            tc.tile_pool(name="attbp", bufs=4) as attbp,
            tc.tile_pool(name="accs", bufs=1) as accs,
            tc.tile_pool(name="small", bufs=12) as small,
            tc.tile_pool(name="dram", bufs=1, space="DRAM") as dram,
        ):
            # ---- load constants into SBUF (one DMA each where possible)
            wvsb = consts.tile([128, 2, 2, 128], BF16)
            nc.sync.dma_start(out=wvsb, in_=wv.rearrange("a b p m -> p a b m"))
            wusb = consts.tile([128, 2, 2, 128], BF16)
            nc.sync.dma_start(out=wusb, in_=wu.rearrange("a b p m -> p a b m"))
            wa1sb = consts.tile([128, 2, H], BF16)
            nc.sync.dma_start(out=wa1sb, in_=wa1.rearrange("a p m -> p a m"))
            wa2sb = consts.tile([H, 1], BF16)
            nc.sync.dma_start(out=wa2sb, in_=wa2)
            b2sb = consts.tile([128, 2, J], F32)
            nc.sync.dma_start(out=b2sb, in_=bias2v)
            bnwsb = consts.tile([1, J], F32)
            nc.sync.dma_start(out=bnwsb, in_=bnw)
            bnbsb = consts.tile([1, J], F32)
            nc.sync.dma_start(out=bnbsb, in_=bnb)
            ab1sb = consts.tile([H, 1], F32)
            nc.sync.dma_start(out=ab1sb, in_=ab1)
            ab2sb = consts.tile([1, 1], F32)
            nc.sync.dma_start(out=ab2sb, in_=ab2)

            # BN stat accumulators: per (channel, joint) partial sums, per chunk
            acc_s = [accs.tile([128, J], F32, name=f"acc_s{q}") for q in range(2)]
            acc_q = [accs.tile([128, J], F32, name=f"acc_q{q}") for q in range(2)]
            for q in range(2):
                nc.vector.memset(acc_s[q], 0.0)
                nc.vector.memset(acc_q[q], 0.0)

            def drain_barrier():
                """strict_bb_all_engine_barrier, but carried by a Drain
                instruction (its ISA struct accepts many sync waits; the
                barrier NoOp's CTRL struct does not)."""
                curr_bb = nc.cur_bb
                assert curr_bb is not None
                prior = list(curr_bb.bb.instructions)
                bi = nc.sync.drain()
                tc.barrier_instruction_and_bb = (bi.ins, curr_bb)
                if (
                    tc.no_sync_barrier_and_bb is not None
                    and tc.no_sync_barrier_and_bb[1] == curr_bb
                ):
                    tc.no_sync_barrier_and_bb = None
                for instruction in prior:
                    tile.add_dep_helper(
                        bi.ins,
                        instruction,
                        sync=bass.sync_unless_reorderable_target(
                            instruction, instruction.is_executable()
                        ),
                        reason="drain_barrier: backward edge",
                    )

            # consts loaded via many DMA queues; join all clocks once so the
            # first readers don't inherit multi-queue waits
            drain_barrier()

            def window(iw, phase):
                """phase 0: accumulate BN stats. phase 1: produce output."""
                if phase == 0:
                    # cast-DMA straight to bf16 (SWDGE), one DMA per window
                    xbf = xbfp.tile([128, 2, J, W], BF16, name="xbf", tag="xbf")
                    nc.gpsimd.dma_start(out=xbf, in_=xt[iw])
                    xf = None
                else:
                    xf = xfp.tile([128, 2, J, W], F32, name="xf", tag="xf")
                    nc.sync.dma_start(out=xf, in_=xt[iw])
                    xbf = xbfp.tile([128, 2, J, W], BF16, name="xbf", tag="xbf")
                    nc.vector.tensor_copy(out=xbf, in_=xf)

                # ---- phase A: vx for all (q, j), drained to SBUF via DVE
                vxsb = {}
                for q in range(2):
                    for j in range(J):
                        ps = vxp.tile([128, W], F32, name="vx_ps", tag="vxp")
                        nc.tensor.matmul(ps, wvsb[:, 0, q, :], xbf[:, 0, j, :],
                                         start=True, stop=False)
                        nc.tensor.matmul(ps, wvsb[:, 1, q, :], xbf[:, 1, j, :],
                                         start=False, stop=True)
                        vs = vxs.tile([128, W], F32, name="vx_sb", tag="vxs")
                        nc.vector.tensor_copy(out=vs, in_=ps)
                        vxsb[(q, j)] = vs

                # ---- phase B (per joint): ux + mix (+stats | +bn/relu/att/out)
                for j in range(J):
                    ks = CONNECTIONS[j]
                    if phase == 1:
                        oj = osbp.tile([128, 2, W], F32, name="oj", tag="oj")
                    for q in range(2):
                        pu = uxp.tile([128, W], F32, name="ux_ps", tag="uxp")
                        nc.tensor.matmul(pu, wusb[:, 0, q, :], xbf[:, 0, j, :],
                                         start=True, stop=False)
                        nc.tensor.matmul(pu, wusb[:, 1, q, :], xbf[:, 1, j, :],
                                         start=False, stop=True)

                        t1 = ytmp.tile([128, W], F32, name="t1", tag="yt")
                        nc.vector.scalar_tensor_tensor(
                            out=t1,
                            in0=vxsb[(q, ks[0])],
                            scalar=float(_ADJ[j, ks[0]]),
                            in1=pu,
                            op0=mybir.AluOpType.mult,
                            op1=mybir.AluOpType.add,
                        )
                        if len(ks) == 2:
                            t2 = ytmp.tile([128, W], F32, name="t2", tag="yt")
                            nc.vector.scalar_tensor_tensor(
                                out=t2,
                                in0=vxsb[(q, ks[1])],
                                scalar=float(_ADJ[j, ks[1]]),
                                in1=t1,
                                op0=mybir.AluOpType.mult,
                                op1=mybir.AluOpType.add,
                            )
                        else:
                            t2 = t1

                        if phase == 0:
                            yb = ybp.tile([128, W], F32, name="yb", tag="yb")
                            tmp1 = small.tile([128, 1], F32, name="tmp1", tag="sm")
                            nc.vector.tensor_scalar(
                                out=yb,
                                in0=t2,
                                scalar1=b2sb[:, q, j:j + 1],
                                scalar2=0.0,
                                op0=mybir.AluOpType.add,
                                op1=mybir.AluOpType.add,
                                accum_out=tmp1,
                            )
                            nc.vector.tensor_tensor(
                                out=acc_s[q][:, j:j + 1],
                                in0=acc_s[q][:, j:j + 1],
                                in1=tmp1,
                                op=mybir.AluOpType.add,
                            )
                            ysq = ysqp.tile([128, W], F32, name="ysq", tag="ysq")
                            tmp2 = small.tile([128, 1], F32, name="tmp2", tag="sm")
                            nc.scalar.activation(
                                out=ysq,
                                in_=yb,
                                func=mybir.ActivationFunctionType.Square,
                                accum_out=tmp2,
                            )
                            nc.vector.tensor_tensor(
                                out=acc_q[q][:, j:j + 1],
                                in0=acc_q[q][:, j:j + 1],
                                in1=tmp2,
                                op=mybir.AluOpType.add,
                            )
                        else:
                            yb = ybp.tile([128, W], F32, name="yb", tag="yb")
                            nc.vector.tensor_scalar(
                                out=yb,
                                in0=t2,
                                scalar1=b2sb[:, q, j:j + 1],
                                scalar2=None,
                                op0=mybir.AluOpType.add,
                            )
                            # z = shat[j]*yb + x ; o = relu(z + bhat[j])
                            z = ytmp.tile([128, W], F32, name="z", tag="yt")
                            nc.vector.scalar_tensor_tensor(
                                out=z,
                                in0=yb,
                                scalar=srep[:, j:j + 1],
                                in1=xf[:, q, j, :],
                                op0=mybir.AluOpType.mult,
                                op1=mybir.AluOpType.add,
                            )
                            nc.vector.tensor_scalar(
                                out=oj[:, q, :],
                                in0=z,
                                scalar1=bhrep[:, j:j + 1],
                                scalar2=0.0,
                                op0=mybir.AluOpType.add,
                                op1=mybir.AluOpType.max,
                            )

                    if phase == 1:
                        # attention gate for joint j, final mul, store
                        ob = obfp.tile([128, 2, W], BF16, name="ob", tag="ob")
                        nc.vector.tensor_copy(out=ob, in_=oj)
                        hp = vxp.tile([H, W], F32, name="h_ps", tag="vxp")
                        nc.tensor.matmul(hp, wa1sb[:, 0, :], ob[:, 0, :],
                                         start=True, stop=False)
                        nc.tensor.matmul(hp, wa1sb[:, 1, :], ob[:, 1, :],
                                         start=False, stop=True)
                        hs = hbfp.tile([H, W], BF16, name="h_sb", tag="hbf")
                        nc.scalar.activation(
                            out=hs,
                            in_=hp,
                            func=mybir.ActivationFunctionType.Relu,
                            bias=ab1sb,
                            scale=1.0,
                        )
                        ap_ = uxp.tile([1, W], F32, name="a_ps", tag="uxp")
                        nc.tensor.matmul(ap_, wa2sb, hs, start=True, stop=True)
                        att = small.tile([1, W], F32, name="att", tag="att")
                        nc.scalar.activation(
                            out=att,
                            in_=ap_,
                            func=mybir.ActivationFunctionType.Sigmoid,
                            bias=ab2sb,
                            scale=1.0,
                        )
                        attb = attbp.tile([128, W], F32, name="attb", tag="attb")
                        nc.gpsimd.partition_broadcast(
                            out_ap=attb, in_ap=att, channels=128
                        )
                        for q in range(2):
                            nc.vector.tensor_tensor(
                                out=oj[:, q, :],
                                in0=oj[:, q, :],
                                in1=attb,
                                op=mybir.AluOpType.mult,
                            )
                        nc.sync.dma_start(
                            out=out_t[iw, :, :, j, :], in_=oj
                        )

            # ================= pass 1: stats =================
            for iw in range(NW):
                window(iw, phase=0)

            drain_barrier()

            # ---- combine stats across partitions, chunks, cores
            par_s = [accs.tile([128, J], F32, name=f"par_s{q}") for q in range(2)]
            par_q = [accs.tile([128, J], F32, name=f"par_q{q}") for q in range(2)]
            for q in range(2):
                nc.gpsimd.partition_all_reduce(
                    out_ap=par_s[q][:, :],
                    in_ap=acc_s[q][:, :],
                    channels=128,
                    reduce_op=bass_isa.ReduceOp.add,
                )
                nc.gpsimd.partition_all_reduce(
                    out_ap=par_q[q][:, :],
                    in_ap=acc_q[q][:, :],
                    channels=128,
                    reduce_op=bass_isa.ReduceOp.add,
                )
            packed = small.tile([1, 2 * J], F32, tag="pk")
            nc.vector.tensor_tensor(
                out=packed[:, 0:J],
                in0=par_s[0][0:1, :],
                in1=par_s[1][0:1, :],
                op=mybir.AluOpType.add,
            )
            nc.vector.tensor_tensor(
                out=packed[:, J:2 * J],
                in0=par_q[0][0:1, :],
                in1=par_q[1][0:1, :],
                op=mybir.AluOpType.add,
            )

            cc_in = dram.tile([1, 2 * J], F32)
            cc_out = dram.tile([1, 2 * J], F32)
            nc.gpsimd.dma_start(out=cc_in, in_=packed)
            nc.gpsimd.collective_compute(
                "AllReduce",
                mybir.AluOpType.add,
                replica_groups=[list(range(NCORES))],
                ins=[cc_in.opt()],
                outs=[cc_out.opt()],
            )
            stats = small.tile([1, 2 * J], F32, tag="pk")
            nc.gpsimd.dma_start(out=stats, in_=cc_out)

            # ---- mu, var, shat = bnw*rsqrt(var+eps), bhat = bnb - mu*shat
            mu = small.tile([1, J], F32, tag="st")
            nc.vector.tensor_scalar(
                out=mu, in0=stats[:, 0:J], scalar1=1.0 / NGLOB, scalar2=None,
                op0=mybir.AluOpType.mult,
            )
            ey2 = small.tile([1, J], F32, tag="st")
            nc.vector.tensor_scalar(
                out=ey2, in0=stats[:, J:2 * J], scalar1=1.0 / NGLOB, scalar2=None,
                op0=mybir.AluOpType.mult,
            )
            mu2 = small.tile([1, J], F32, tag="st")
            nc.vector.tensor_tensor(out=mu2, in0=mu, in1=mu, op=mybir.AluOpType.mult)
            var = small.tile([1, J], F32, tag="st")
            nc.vector.tensor_tensor(out=var, in0=ey2, in1=mu2,
                                    op=mybir.AluOpType.subtract)
            epssb = small.tile([1, 1], F32, tag="st")
            nc.vector.memset(epssb, EPS)
            sd = small.tile([1, J], F32, tag="st")
            nc.scalar.activation(
                out=sd, in_=var, func=mybir.ActivationFunctionType.Sqrt,
                bias=epssb, scale=1.0,
            )
            rstd = small.tile([1, J], F32, tag="st")
            nc.vector.reciprocal(out=rstd, in_=sd)
            shat = small.tile([1, J], F32, tag="st")
            nc.vector.tensor_tensor(out=shat, in0=bnwsb, in1=rstd,
                                    op=mybir.AluOpType.mult)
            bhat = small.tile([1, J], F32, tag="st")
            nc.vector.tensor_tensor(out=bhat, in0=mu, in1=shat,
                                    op=mybir.AluOpType.mult)
            nc.vector.tensor_tensor(out=bhat, in0=bnbsb, in1=bhat,
                                    op=mybir.AluOpType.subtract)
            srep = consts.tile([128, J], F32)
            nc.gpsimd.partition_broadcast(out_ap=srep, in_ap=shat, channels=128)
            bhrep = consts.tile([128, J], F32)
            nc.gpsimd.partition_broadcast(out_ap=bhrep, in_ap=bhat, channels=128)

            # join clocks again before the apply pass
            drain_barrier()

            # ================= pass 2: apply =================
            for iw in range(NW):
                window(iw, phase=1)

    nc.compile()
    return nc


_CACHE: dict = {}


def _host_inputs(x, U_w, U_b, V_w, V_b, bn_w, bn_b, att_w1, att_b1, att_w2, att_b2):
    """Build the per-core input maps."""
    f32 = np.float32
    bf16 = ml_dtypes.bfloat16
    xtf = np.ascontiguousarray(x.transpose(3, 2, 0, 1))  # [C, J, B, T]

    def chunks22(wT):  # [C,C] (c_in x c_out) -> [2,2,128,128] bf16
        return np.ascontiguousarray(
            wT.reshape(2, 128, 2, 128).transpose(0, 2, 1, 3)
        ).astype(bf16)

    wv = chunks22(np.ascontiguousarray(V_w.T).astype(f32))
    wu = chunks22(np.ascontiguousarray(U_w.T).astype(f32))
    wa1 = np.ascontiguousarray(att_w1.T.reshape(2, 128, H)).astype(bf16)
    wa2 = np.ascontiguousarray(att_w2.T).astype(bf16)        # [H,1]
    rowsum = _ADJ.sum(axis=1)                                 # [J]
    bias2 = (rowsum[None, :] * V_b[:, None] + U_b[:, None]).astype(f32)  # [C,J]
    bnw = bn_w.reshape(1, J).astype(f32)
    bnb = bn_b.reshape(1, J).astype(f32)
    ab1 = att_b1.reshape(H, 1).astype(f32)
    ab2 = att_b2.reshape(1, 1).astype(f32)

    shared = dict(wv=wv, wu=wu, wa1=wa1, wa2=wa2, bias2=bias2, bnw=bnw,
                  bnb=bnb, ab1=ab1, ab2=ab2)
    in_maps = []
    for i in range(NCORES):
        xt_i = np.ascontiguousarray(
            xtf[:, :, i * BPC:(i + 1) * BPC, :]
        ).reshape(2, 128, J, NW, W)
        xt_i = np.ascontiguousarray(xt_i.transpose(3, 1, 0, 2, 4))
        in_maps.append(dict(xt=xt_i, **shared))
    return in_maps


def kernel(x, U_w, U_b, V_w, V_b, bn_w, bn_b, att_w1, att_b1, att_w2, att_b2,
           _trace=False):
    x = np.asarray(x, dtype=np.float32)
    args = [np.asarray(a, dtype=np.float32)
            for a in (U_w, U_b, V_w, V_b, bn_w, bn_b, att_w1, att_b1, att_w2,
                      att_b2)]
    in_maps = _host_inputs(x, *args)

    if "nc" not in _CACHE:
        _CACHE["nc"] = _build_program()
    nc = _CACHE["nc"]

    res = run_bass_kernel_spmd(nc, in_maps, list(range(NCORES)), trace=_trace)
    _CACHE["last_results"] = res

    # out_t per core: [NW, 128, 2, J, W] -> [C, J, NBT] -> [B,T,J,C]
    outs = []
    for i in range(NCORES):
        o = res.results[i]["out_t"].transpose(2, 1, 3, 0, 4).reshape(C, J, BPC, T)
        outs.append(o)
    full = np.stack(outs)                       # [8, C, J, BPC, T]
    out = full.transpose(0, 3, 4, 2, 1).reshape(B, T, J, C)
    return np.ascontiguousarray(out)



# revision 12
# speedup vs baseline: 2.9749x; 2.9749x over previous
"""Trainium2 Bass kernel for the GCN message-passing block (nn_Model_16217796510271).

Contract: kernel(**inputs) takes FULL fp32 inputs (x: [64,243,17,256] + weights)
and returns the FULL fp32 output [64,243,17,256]. Batch axis sharded 8 ways.

Design (v2, overhead-minimized):
- Host pre-casts x to bf16 in a [NW, 128, 2, J, W] channels-on-partitions
  layout with W=486-wide column windows (NW=4, 1944 (b,t) columns/core).
- The normalized adjacency is folded into the matmuls: y_j accumulates in
  PSUM as  U x_j + V^{s_j} m_j  where m_j is a single-STT pre-mix of the
  neighbor columns and V^{s} are <=3 pre-scaled copies of V. No per-joint
  vector mixing, no PSUM->SBUF vx drains.
- Pass 0: ACT drains y (bias fused) to bf16 with accum_out giving per-
  (chunk,joint,window) sums; DVE tensor_tensor_reduce gives sum-of-squares.
  y goes to an HBM scratch in bf16; x stays resident in SBUF (132 KB/part).
- Stats: tiny reductions + gpsimd partition_all_reduce + one [1,34]
  AllReduce across the 8 cores; shat/bhat broadcast to [128,17].
- Pass 1: reread y, STT (s*y + x) + TS (bias, relu) on DVE, joint-attention
  via PE (att2 uses a replicated-w2 stationary so the per-column gate lands
  broadcast across 128 partitions in PSUM), sigmoid on ACT, final gate
  multiply on GPSIMD, bf16 output DMA.
"""

import sys

for _p in ("/opt/trn_rl_repo",):
    if _p not in sys.path:
        sys.path.insert(0, _p)

import ml_dtypes
import numpy as np

import concourse.bacc as bacc
import concourse.bass as bass
import concourse.tile as tile
from concourse import bass_isa, mybir
from concourse.bass_utils import run_bass_kernel_spmd

# ---------------------------------------------------------------- constants
CONNECTIONS = {
    10: [9], 9: [8, 10], 8: [7, 9], 14: [15, 8], 15: [16, 14], 11: [12, 8],
    12: [13, 11], 7: [0, 8], 0: [1, 7], 1: [2, 0], 2: [3, 1], 4: [5, 0],
    5: [6, 4], 16: [15], 13: [12], 3: [2], 6: [5],
}
J = 17
C = 256
H = 64
B = 64
T = 243
EPS = 1e-5

NCORES = 8
BPC = B // NCORES
NBT = BPC * T                # 1944
W = 486                      # window width (<=512 PSUM fp32 bank)
NW = NBT // W                # 4
NGLOB = B * T * C            # BN count per joint

F32 = mybir.dt.float32
BF16 = mybir.dt.bfloat16


def _norm_adj() -> np.ndarray:
    adj = np.zeros((J, J), dtype=np.float32)
    for i, ks in CONNECTIONS.items():
        for k in ks:
            adj[i, k] = 1.0
    dinv = adj.sum(-1) ** -0.5
    return (dinv[:, None] * adj * dinv[None, :]).astype(np.float32)


_ADJ = _norm_adj()


def _fold_adjacency():
    """Per joint j choose (ka, kb, r, s) with
       sum_k A[j,k] V x_k == V^{s}( r*x_ka + x_kb ),  V^{s} = s*V.
    deg-1 joints use (None, k0, None, A[j,k0]) (moving = x_k0 directly).
    Choose kb per deg-2 joint to minimize the number of distinct s values.
    """
    deg2 = [(j, ks) for j, ks in CONNECTIONS.items() if len(ks) == 2]
    deg1 = [(j, ks[0]) for j, ks in CONNECTIONS.items() if len(ks) == 1]
    base = {round(float(_ADJ[j, k]), 6) for j, k in deg1}
    best = None
    for mask in range(1 << len(deg2)):
        vals = set(base)
        for i, (j, ks) in enumerate(deg2):
            kb = ks[(mask >> i) & 1]
            vals.add(round(float(_ADJ[j, kb]), 6))
        if best is None or len(vals) < len(best[1]):
            best = (mask, vals)
        if len(vals) == 1:
            break
    mask = best[0]
    variants = sorted(best[1])
    vidx = {v: i for i, v in enumerate(variants)}
    plan = {}
    for j, k in deg1:
        s = round(float(_ADJ[j, k]), 6)
        plan[j] = (None, k, None, vidx[s])
    for i, (j, ks) in enumerate(deg2):
        kb = ks[(mask >> i) & 1]
        ka = ks[1 - ((mask >> i) & 1)]
        s = round(float(_ADJ[j, kb]), 6)
        r = float(_ADJ[j, ka]) / float(_ADJ[j, kb])
        plan[j] = (ka, kb, r, vidx[s])
    return plan, np.array(variants, dtype=np.float32)


_PLAN, _VARIANTS = _fold_adjacency()
NVAR = len(_VARIANTS)

# joint groups for PSUM pipelining (both q of a group in flight <= 8 banks)
_JGROUPS = [[0, 1, 2, 3], [4, 5, 6, 7], [8, 9, 10, 11], [12, 13, 14],
            [15, 16]]


# ---------------------------------------------------------------- device program
def _build_program() -> bass.Bass:
    nc = bacc.Bacc(
        "TRN2",
        target_bir_lowering=False,
        debug=False,
        num_devices=NCORES,
    )
    AF = mybir.ActivationFunctionType
    OP = mybir.AluOpType

    xt = nc.dram_tensor("xt", [NW, 128, 2, J, W], BF16, kind="ExternalInput").ap()
    wu = nc.dram_tensor("wu", [2, 2, 128, 128], BF16, kind="ExternalInput").ap()
    wv = nc.dram_tensor("wv", [NVAR, 2, 2, 128, 128], BF16,
                        kind="ExternalInput").ap()
    wa1 = nc.dram_tensor("wa1", [2, 128, H], BF16, kind="ExternalInput").ap()
    w2r = nc.dram_tensor("w2r", [H, 128], BF16, kind="ExternalInput").ap()
    b2 = nc.dram_tensor("b2", [128, 2, J], F32, kind="ExternalInput").ap()
    bnw = nc.dram_tensor("bnw", [1, J], F32, kind="ExternalInput").ap()
    bnb = nc.dram_tensor("bnb", [1, J], F32, kind="ExternalInput").ap()
    ab1 = nc.dram_tensor("ab1", [H, 1], F32, kind="ExternalInput").ap()
    ab2r = nc.dram_tensor("ab2r", [128, 1], F32, kind="ExternalInput").ap()
    out_t = nc.dram_tensor("out_t", [NW, J, 128, 2, W], BF16,
                           kind="ExternalOutput").ap()

    with tile.TileContext(nc) as tc:
        with (
            tc.tile_pool(name="consts", bufs=1) as consts,
            tc.tile_pool(name="xp", bufs=1) as xp,
            tc.tile_pool(name="mp", bufs=8) as mp,
            tc.tile_pool(name="psp", bufs=8, space="PSUM") as psp,
            tc.tile_pool(name="ysp", bufs=2) as ysp,
            tc.tile_pool(name="scp", bufs=2) as scp,
            tc.tile_pool(name="zp", bufs=3) as zp,
            tc.tile_pool(name="obp", bufs=3) as obp,
            tc.tile_pool(name="hp", bufs=2) as hp,
            tc.tile_pool(name="abp", bufs=3) as abp,
            tc.tile_pool(name="stat", bufs=1) as stat,
            tc.tile_pool(name="small", bufs=1) as small,
            tc.tile_pool(name="dram", bufs=1, space="DRAM") as dram,
        ):
            # ---- constants
            wusb = consts.tile([128, 2, 2, 128], BF16)
            nc.sync.dma_start(out=wusb, in_=wu.rearrange("a q k m -> k a q m"))
            wvsb = consts.tile([128, NVAR, 2, 2, 128], BF16)
            nc.sync.dma_start(out=wvsb, in_=wv.rearrange("v a q k m -> k v a q m"))
            wa1sb = consts.tile([128, 2, H], BF16)
            nc.sync.dma_start(out=wa1sb, in_=wa1.rearrange("a k h -> k a h"))
            w2rsb = consts.tile([H, 128], BF16)
            nc.sync.dma_start(out=w2rsb, in_=w2r)
            b2sb = consts.tile([128, 2, J], F32)
            nc.sync.dma_start(out=b2sb, in_=b2)
            bnwsb = consts.tile([1, J], F32)
            nc.sync.dma_start(out=bnwsb, in_=bnw)
            bnbsb = consts.tile([1, J], F32)
            nc.sync.dma_start(out=bnbsb, in_=bnb)
            ab1sb = consts.tile([H, 1], F32)
            nc.sync.dma_start(out=ab1sb, in_=ab1)
            ab2rsb = consts.tile([128, 1], F32)
            nc.sync.dma_start(out=ab2rsb, in_=ab2r)

            # bn_stats output per (q*J+j, w): [cnt_e, mean_e, cnt*var_e,
            # cnt_o, mean_o, cnt*var_o], counts are W/2 = 243
            bns = stat.tile([128, 2 * J, NW, 6], F32, name="bns")

            # persistent x tiles, one per (window, chunk)
            x_sb = {}
            for w in range(NW):
                for a in range(2):
                    t = xp.tile([128, J, W], BF16, name=f"x_{w}_{a}")
                    nc.sync.dma_start(out=t[:, 0:9, :], in_=xt[w][:, a, 0:9, :])
                    nc.sync.dma_start(out=t[:, 9:J, :], in_=xt[w][:, a, 9:J, :])
                    x_sb[(w, a)] = t

            y_hbm = dram.tile([NW, 2, 128, J, W], BF16)

            def drain_barrier():
                curr_bb = nc.cur_bb
                assert curr_bb is not None
                prior = list(curr_bb.bb.instructions)
                bi = nc.sync.drain()
                tc.barrier_instruction_and_bb = (bi.ins, curr_bb)
                if (
                    tc.no_sync_barrier_and_bb is not None
                    and tc.no_sync_barrier_and_bb[1] == curr_bb
                ):
                    tc.no_sync_barrier_and_bb = None
                for instruction in prior:
                    tile.add_dep_helper(
                        bi.ins,
                        instruction,
                        sync=bass.sync_unless_reorderable_target(
                            instruction, instruction.is_executable()
                        ),
                        reason="drain_barrier: backward edge",
                    )

            drain_barrier()

            # ================= pass 0: y + stats =================
            for w in range(NW):
                # neighbor pre-mix tiles (deg-2 joints)
                mt = {}
                for j in range(J):
                    ka, kb, r, v = _PLAN[j]
                    if ka is None:
                        continue
                    for a in range(2):
                        m = mp.tile([128, W], BF16, name="m", tag="m")
                        nc.vector.scalar_tensor_tensor(
                            out=m,
                            in0=x_sb[(w, a)][:, ka, :],
                            scalar=float(r),
                            in1=x_sb[(w, a)][:, kb, :],
                            op0=OP.mult,
                            op1=OP.add,
                        )
                        mt[(a, j)] = m

                stage = {
                    q: ysp.tile([128, J, W], BF16, name="ystage", tag="ys")
                    for q in range(2)
                }
                for grp in _JGROUPS:
                    for q in range(2):
                        ps = {}
                        for j in grp:
                            ps[j] = psp.tile([128, W], F32, name="yps", tag="ps")
                        # U matmuls (stationary reused across the group)
                        for a in range(2):
                            for j in grp:
                                nc.tensor.matmul(
                                    ps[j], wusb[:, a, q, :],
                                    x_sb[(w, a)][:, j, :],
                                    start=(a == 0), stop=False,
                                )
                        # V^{s} matmuls, grouped by variant
                        for a in range(2):
                            for j in sorted(grp, key=lambda j: _PLAN[j][3]):
                                ka, kb, r, v = _PLAN[j]
                                mv = mt[(a, j)] if ka is not None \
                                    else x_sb[(w, a)][:, kb, :]
                                nc.tensor.matmul(
                                    ps[j], wvsb[:, v, a, q, :], mv,
                                    start=False, stop=(a == 1),
                                )
                        for j in grp:
                            col = q * J + j
                            nc.scalar.activation(
                                out=stage[q][:, j, :],
                                in_=ps[j],
                                func=AF.Identity,
                                bias=b2sb[:, q, j:j + 1],
                                scale=1.0,
                            )
                            nc.vector.bn_stats(
                                out=bns[:, col, w],
                                in_=stage[q][:, j, :],
                            )
                for q in range(2):
                    nc.sync.dma_start(out=y_hbm[w, q][:, 0:9, :],
                                      in_=stage[q][:, 0:9, :])
                    nc.sync.dma_start(out=y_hbm[w, q][:, 9:J, :],
                                      in_=stage[q][:, 9:J, :])

            drain_barrier()

            # ---- reduce stats: over windows, chunks, partitions, cores
            # sums (scaled by 1/243): ms = mean_e + mean_o
            # sumsq: cv_e + cv_o + 243*(mean_e^2 + mean_o^2)
            HW2 = float(W // 2)
            me = bns[:, :, :, 1]
            mo = bns[:, :, :, 4]
            cve = bns[:, :, :, 2]
            cvo = bns[:, :, :, 5]
            ms = stat.tile([128, 2 * J, NW], F32, name="ms")
            nc.vector.tensor_tensor(out=ms, in0=me, in1=mo, op=OP.add)
            cv = stat.tile([128, 2 * J, NW], F32, name="cv")
            nc.vector.tensor_tensor(out=cv, in0=cve, in1=cvo, op=OP.add)
            m2e = stat.tile([128, 2 * J, NW], F32, name="m2e")
            nc.vector.tensor_tensor(out=m2e, in0=me, in1=me, op=OP.mult)
            m2o = stat.tile([128, 2 * J, NW], F32, name="m2o")
            nc.vector.tensor_tensor(out=m2o, in0=mo, in1=mo, op=OP.mult)
            m2s = stat.tile([128, 2 * J, NW], F32, name="m2s")
            nc.vector.tensor_tensor(out=m2s, in0=m2e, in1=m2o, op=OP.add)
            sqc = stat.tile([128, 2 * J, NW], F32, name="sqc")
            nc.vector.scalar_tensor_tensor(
                out=sqc, in0=m2s, scalar=HW2, in1=cv, op0=OP.mult, op1=OP.add)

            red_s = small.tile([128, 2 * J], F32)
            red_q = small.tile([128, 2 * J], F32)
            for wide, red in ((ms, red_s), (sqc, red_q)):
                t01 = small.tile([128, 2 * J], F32, name="t01")
                nc.vector.tensor_tensor(out=t01, in0=wide[:, :, 0],
                                        in1=wide[:, :, 1], op=OP.add)
                t23 = small.tile([128, 2 * J], F32, name="t23")
                nc.vector.tensor_tensor(out=t23, in0=wide[:, :, 2],
                                        in1=wide[:, :, 3], op=OP.add)
                nc.vector.tensor_tensor(out=red, in0=t01, in1=t23, op=OP.add)
            sj = small.tile([128, J], F32)
            qj = small.tile([128, J], F32)
            nc.vector.tensor_tensor(out=sj, in0=red_s[:, 0:J],
                                    in1=red_s[:, J:2 * J], op=OP.add)
            nc.vector.tensor_tensor(out=qj, in0=red_q[:, 0:J],
                                    in1=red_q[:, J:2 * J], op=OP.add)
            par_s = small.tile([128, J], F32)
            par_q = small.tile([128, J], F32)
            nc.gpsimd.partition_all_reduce(
                out_ap=par_s, in_ap=sj, channels=128,
                reduce_op=bass_isa.ReduceOp.add)
            nc.gpsimd.partition_all_reduce(
                out_ap=par_q, in_ap=qj, channels=128,
                reduce_op=bass_isa.ReduceOp.add)
            packed = small.tile([1, 2 * J], F32)
            nc.vector.tensor_copy(out=packed[:, 0:J], in_=par_s[0:1, :])
            nc.vector.tensor_copy(out=packed[:, J:2 * J], in_=par_q[0:1, :])

            cc_in = dram.tile([1, 2 * J], F32)
            cc_out = dram.tile([1, 2 * J], F32)
            nc.gpsimd.dma_start(out=cc_in, in_=packed)
            nc.gpsimd.collective_compute(
                "AllReduce",
                OP.add,
                replica_groups=[list(range(NCORES))],
                ins=[cc_in.opt()],
                outs=[cc_out.opt()],
            )
            stats = small.tile([1, 2 * J], F32)
            nc.gpsimd.dma_start(out=stats, in_=cc_out)

            mu = small.tile([1, J], F32)
            nc.vector.tensor_scalar(
                out=mu, in0=stats[:, 0:J], scalar1=HW2 / NGLOB, scalar2=None,
                op0=OP.mult)
            ey2 = small.tile([1, J], F32)
            nc.vector.tensor_scalar(
                out=ey2, in0=stats[:, J:2 * J], scalar1=1.0 / NGLOB,
                scalar2=None, op0=OP.mult)
            mu2 = small.tile([1, J], F32)
            nc.vector.tensor_tensor(out=mu2, in0=mu, in1=mu, op=OP.mult)
            var = small.tile([1, J], F32)
            nc.vector.tensor_tensor(out=var, in0=ey2, in1=mu2, op=OP.subtract)
            epssb = small.tile([1, 1], F32)
            nc.vector.memset(epssb, EPS)
            sd = small.tile([1, J], F32)
            nc.scalar.activation(out=sd, in_=var, func=AF.Sqrt, bias=epssb,
                                 scale=1.0)
            rstd = small.tile([1, J], F32)
            nc.vector.reciprocal(out=rstd, in_=sd)
            shat = small.tile([1, J], F32)
            nc.vector.tensor_tensor(out=shat, in0=bnwsb, in1=rstd, op=OP.mult)
            bhat = small.tile([1, J], F32)
            nc.vector.tensor_tensor(out=bhat, in0=mu, in1=shat, op=OP.mult)
            nc.vector.tensor_tensor(out=bhat, in0=bnbsb, in1=bhat,
                                    op=OP.subtract)
            srep = consts.tile([128, J], F32)
            nc.gpsimd.partition_broadcast(out_ap=srep, in_ap=shat, channels=128)
            bhrep = consts.tile([128, J], F32)
            nc.gpsimd.partition_broadcast(out_ap=bhrep, in_ap=bhat,
                                          channels=128)

            drain_barrier()

            # ================= pass 1: apply =================
            for w in range(NW):
                yrd = {}
                for q in range(2):
                    t = ysp.tile([128, J, W], BF16, name="yrd", tag="ys")
                    nc.sync.dma_start(out=t[:, 0:9, :],
                                      in_=y_hbm[w, q][:, 0:9, :])
                    nc.sync.dma_start(out=t[:, 9:J, :],
                                      in_=y_hbm[w, q][:, 9:J, :])
                    yrd[q] = t
                for j in range(J):
                    ob = obp.tile([128, 2, W], BF16, name="ob", tag="ob")
                    for q in range(2):
                        z = zp.tile([128, W], BF16, name="z", tag="z")
                        nc.vector.scalar_tensor_tensor(
                            out=z,
                            in0=yrd[q][:, j, :],
                            scalar=srep[:, j:j + 1],
                            in1=x_sb[(w, q)][:, j, :],
                            op0=OP.mult,
                            op1=OP.add,
                        )
                        nc.vector.tensor_scalar(
                            out=ob[:, q, :],
                            in0=z,
                            scalar1=bhrep[:, j:j + 1],
                            scalar2=0.0,
                            op0=OP.add,
                            op1=OP.max,
                        )
                    hps = psp.tile([H, W], F32, name="hps", tag="ps")
                    nc.tensor.matmul(hps, wa1sb[:, 0, :], ob[:, 0, :],
                                     start=True, stop=False)
                    nc.tensor.matmul(hps, wa1sb[:, 1, :], ob[:, 1, :],
                                     start=False, stop=True)
                    hbf = hp.tile([H, W], BF16, name="hbf", tag="h")
                    nc.scalar.activation(out=hbf, in_=hps, func=AF.Relu,
                                         bias=ab1sb, scale=1.0)
                    aps = psp.tile([128, W], F32, name="aps", tag="ps")
                    nc.tensor.matmul(aps, w2rsb, hbf, start=True, stop=True)
                    attb = abp.tile([128, W], BF16, name="attb", tag="attb")
                    nc.scalar.activation(out=attb, in_=aps, func=AF.Sigmoid,
                                         bias=ab2rsb, scale=1.0)
                    for q in range(2):
                        nc.vector.tensor_tensor(
                            out=ob[:, q, :], in0=ob[:, q, :], in1=attb,
                            op=OP.mult)
                    nc.sync.dma_start(out=out_t[w, j], in_=ob)

    nc.compile()
    return nc


_CACHE: dict = {}


def _host_inputs(x, U_w, U_b, V_w, V_b, bn_w, bn_b, att_w1, att_b1, att_w2,
                 att_b2):
    f32 = np.float32
    bf16 = ml_dtypes.bfloat16

    def chunks(wT):  # [C(in), M(out)] -> [a, q, 128, 128]
        return np.ascontiguousarray(
            wT.reshape(2, 128, 2, 128).transpose(0, 2, 1, 3))

    uT = np.ascontiguousarray(U_w.T).astype(f32)   # [c_in, c_out]
    vT = np.ascontiguousarray(V_w.T).astype(f32)
    wu = chunks(uT).astype(bf16)
    wv = np.stack([chunks(s * vT) for s in _VARIANTS]).astype(bf16)
    wa1 = np.ascontiguousarray(att_w1.T.reshape(2, 128, H)).astype(bf16)
    w2r = np.ascontiguousarray(
        np.tile(att_w2.T.astype(f32), (1, 128))).astype(bf16)  # [H, 128]
    rowsum = _ADJ.sum(axis=1)
    b2 = (rowsum[None, :] * V_b[:, None] + U_b[:, None]).astype(f32)  # [C, J]
    b2 = np.ascontiguousarray(b2.reshape(2, 128, J).transpose(1, 0, 2))
    bnw = bn_w.reshape(1, J).astype(f32)
    bnb = bn_b.reshape(1, J).astype(f32)
    ab1 = att_b1.reshape(H, 1).astype(f32)
    ab2r = np.ascontiguousarray(
        np.tile(att_b2.reshape(1, 1).astype(f32), (128, 1)))

    shared = dict(wu=wu, wv=wv, wa1=wa1, w2r=w2r, b2=b2, bnw=bnw, bnb=bnb,
                  ab1=ab1, ab2r=ab2r)

    xtf = np.ascontiguousarray(x.transpose(3, 2, 0, 1))  # [C, J, B, T]
    in_maps = []
    for i in range(NCORES):
        xc = xtf[:, :, i * BPC:(i + 1) * BPC, :].reshape(C, J, NBT)
        xc = xc.reshape(2, 128, J, NW, W)
        xc = np.ascontiguousarray(xc.transpose(3, 1, 0, 2, 4)).astype(bf16)
        in_maps.append(dict(xt=xc, **shared))
    return in_maps


def kernel(x, U_w, U_b, V_w, V_b, bn_w, bn_b, att_w1, att_b1, att_w2, att_b2,
           _trace=False):
    x = np.asarray(x, dtype=np.float32)
    args = [np.asarray(a, dtype=np.float32)
            for a in (U_w, U_b, V_w, V_b, bn_w, bn_b, att_w1, att_b1, att_w2,
                      att_b2)]
    in_maps = _host_inputs(x, *args)

    if "nc" not in _CACHE:
        _CACHE["nc"] = _build_program()
    nc = _CACHE["nc"]

    res = run_bass_kernel_spmd(nc, in_maps, list(range(NCORES)), trace=_trace)
    _CACHE["last_results"] = res

    # out_t per core: [NW, J, 128, 2, W] -> [BPC, T, J, C]
    outs = []
    for i in range(NCORES):
        o = res.results[i]["out_t"]                     # bf16
        o = o.transpose(3, 2, 1, 0, 4).reshape(C, J, NBT)
        o = o.transpose(2, 1, 0).reshape(BPC, T, J, C)
        outs.append(o)
    out = np.concatenate(outs, axis=0).astype(np.float32)
    return np.ascontiguousarray(out)
